# revision 21
# baseline (speedup 1.0000x reference)
"""Trainium2 Bass kernel for a 3-layer LSTM (B=64, T=256, F=64, H=1024)
+ tanh output projection, SPMD across 8 NeuronCores.

Flipped-layout version: gate pre-activations are computed as
[128 partitions = h-slice, 4 gate types x B free] with the WEIGHTS as
the stationary matmul operand ([128, 128] tiles) and the gathered h as
the moving operand ([128, B]). Each matmul streams only B=64 rows with
all 128 PE columns active, halving Tensor-engine time vs the
batch-on-partitions layout (which streamed 512 rows with M=64).
The elementwise LSTM cell runs on [128, B] tiles and its final product
writes h directly into the packed exchange buffer - the PE transposes
and DVE staging copies of the old layout disappear entirely.

Everything else (gate-dim sharding, 3-layer software pipeline, one
packed [128, 192] bf16 AllGather per slot, y-projection groups) matches
the baseline.
"""

import sys

sys.path.insert(0, "/opt/trn_rl_repo")

import numpy as np
import ml_dtypes

BF16 = ml_dtypes.bfloat16

B, T, F, H = 64, 256, 64, 1024
R = 8           # cores
HS = H // R     # 128 h slice per core
YG = 8          # y-projection group size (slots)
NB = 4          # gather ring depth
W23 = 3 * B     # packed exchange width (h1|h2|h3)

_GATE_ORDER = (0, 1, 3, 2)  # i, f, o, g (PyTorch row blocks i,f,g,o)


def _wf(w, r):
    """[4H, K] weight -> lhsT tiles [128(kpart), K/128, 4(type), 128(m)]."""
    K = w.shape[1]
    wr = np.stack([w[g * H + HS * r: g * H + HS * (r + 1), :]
                   for g in _GATE_ORDER])          # [4, 128(m), K]
    return np.ascontiguousarray(
        wr.reshape(4, HS, K // 128, 128).transpose(3, 2, 0, 1))


def _prep_core_inputs(r, X, weights):
    (w_ih1, w_hh1, b_ih1, b_hh1, w_ih2, w_hh2, b_ih2, b_hh2,
     w_ih3, w_hh3, b_ih3, b_hh3, w_out, b_out) = weights
    inp = {}
    # X: [B, T, F] -> [T, 128(pad F), B] with constant-1 row at F (bias row)
    Xt = np.zeros((T, 128, B), np.float32)
    Xt[:, :F, :] = X.transpose(1, 2, 0)
    Xt[:, F, :] = 1.0
    inp["Xt"] = Xt.astype(BF16)

    def btypes(bi, bh):
        s = bi + bh
        return np.stack([s[g * H + HS * r: g * H + HS * (r + 1)]
                         for g in _GATE_ORDER])    # [4, 128]

    # layer 1 x-side: lhsT [128(kpart: F data + bias row at F), 4, 128]
    w1r = np.stack([w_ih1[g * H + HS * r: g * H + HS * (r + 1), :]
                    for g in _GATE_ORDER])         # [4, 128, F]
    W1xf = np.zeros((128, 4, HS), np.float32)
    W1xf[:F] = w1r.transpose(2, 0, 1)
    W1xf[F] = btypes(b_ih1, b_hh1)
    inp["W1xf"] = W1xf.astype(BF16)

    for name, w in (("Whh1", w_hh1), ("Wih2", w_ih2), ("Whh2", w_hh2),
                    ("Wih3", w_ih3), ("Whh3", w_hh3)):
        inp[name] = _wf(np.asarray(w, np.float32), r).astype(BF16)

    inp["bias2f"] = btypes(b_ih2, b_hh2).astype(BF16)   # [4, 128]
    inp["bias3f"] = btypes(b_ih3, b_hh3).astype(BF16)
    onehot = np.zeros((4, 4 * B), np.float32)
    for g in range(4):
        onehot[g, g * B:(g + 1) * B] = 1.0
    inp["onehot4"] = onehot.astype(BF16)
    # output projection: w_out [F, H] -> lhsT tiles [128, 8, F]
    woT = np.ascontiguousarray(w_out.T).astype(np.float32)  # [H, F]
    inp["Wout"] = np.ascontiguousarray(
        woT.reshape(8, 128, F).transpose(1, 0, 2)).astype(BF16)
    inp["bout"] = b_out.reshape(F, 1).astype(np.float32)
    return inp


def build_nc(t_steps=T, reps=1, y_small=False):
    import concourse.mybir as mybir
    import concourse.tile as tile
    from concourse import bacc

    f32 = mybir.dt.float32
    bf16 = mybir.dt.bfloat16
    AF = mybir.ActivationFunctionType
    NSLOT = t_steps + 3
    NEX = t_steps + 2   # exchanges per rep (slots 0..T+1)
    rg = [list(range(R))]

    nc = bacc.Bacc("TRN2", target_bir_lowering=False, debug=False, num_devices=R)

    p_Xt = nc.dram_tensor("Xt", [T, 128, B], bf16, kind="ExternalInput")
    p_W1xf = nc.dram_tensor("W1xf", [128, 4, HS], bf16, kind="ExternalInput")
    pw = {}
    for name in ("Whh1", "Wih2", "Whh2", "Wih3", "Whh3"):
        pw[name] = nc.dram_tensor(name, [128, 8, 4, HS], bf16,
                                  kind="ExternalInput")
    p_b2 = nc.dram_tensor("bias2f", [4, HS], bf16, kind="ExternalInput")
    p_b3 = nc.dram_tensor("bias3f", [4, HS], bf16, kind="ExternalInput")
    p_oh = nc.dram_tensor("onehot4", [4, 4 * B], bf16, kind="ExternalInput")
    p_Wout = nc.dram_tensor("Wout", [128, 8, F], bf16, kind="ExternalInput")
    p_bout = nc.dram_tensor("bout", [F, 1], f32, kind="ExternalInput")
    ycols = YG * B if y_small else t_steps * B
    p_Y = nc.dram_tensor("Y", [F, ycols], f32, kind="ExternalOutput")

    with tile.TileContext(nc) as tc:
        with (
            tc.tile_pool(name="wpool", bufs=1) as wpool,
            tc.tile_pool(name="state", bufs=1) as state,
            tc.tile_pool(name="xq", bufs=4) as xq,
            tc.tile_pool(name="sbt", bufs=3) as sbt,
            tc.tile_pool(name="h3g", bufs=2) as h3g,
            tc.tile_pool(name="gps", bufs=6, space="PSUM") as gps,
            tc.tile_pool(name="yps", bufs=1, space="PSUM") as yps,
            tc.tile_pool(name="dms", bufs=4, space="DRAM") as dms,
        ):
            # ---- resident weights ----
            W1xf = wpool.tile([128, 4, HS], bf16, tag="W1xf")
            nc.sync.dma_start(W1xf[:], p_W1xf[:])
            W = {}
            for name in ("Whh1", "Wih2", "Whh2", "Wih3", "Whh3"):
                W[name] = wpool.tile([128, 8, 4, HS], bf16, tag=name,
                                     name=name + "_sb")
                nc.sync.dma_start(W[name][:], pw[name][:])
            b2f = wpool.tile([4, HS], bf16, tag="b2f", name="b2f_sb")
            nc.sync.dma_start(b2f[:], p_b2[:])
            b3f = wpool.tile([4, HS], bf16, tag="b3f", name="b3f_sb")
            nc.sync.dma_start(b3f[:], p_b3[:])
            oh4 = wpool.tile([4, 4 * B], bf16, tag="oh4", name="oh4_sb")
            nc.sync.dma_start(oh4[:], p_oh[:])
            Wout = wpool.tile([128, 8, F], bf16, tag="Wout")
            nc.sync.dma_start(Wout[:], p_Wout[:])
            bout = wpool.tile([F, 1], f32, tag="bout")
            nc.sync.dma_start(bout[:], p_bout[:])

            # ---- exchange buffers ----
            stg = [wpool.tile([128, W23], bf16, tag=f"stg{i}", name=f"stg{i}")
                   for i in range(2)]
            for t_ in stg:
                nc.vector.memset(t_[:], 0.0)
            Hgp = [wpool.tile([128, R, W23], bf16, tag=f"Hgp{i}", name=f"Hgp{i}")
                   for i in range(NB)]

            # ---- persistent state: c per layer [128(h), B] ----
            cts = [state.tile([128, B], f32, tag=f"c{l}", name=f"c{l}")
                   for l in (1, 2, 3)]

            def lstm_ew(key, gp, c, sb, off):
                """gates psum [128, 4B] (i|f|o|g x B) + c [128, B]
                -> h bf16 written to sb[:, off:off+B]."""
                sio = sbt.tile([128, 3 * B], f32, tag=f"sio{key}",
                               name=f"sio{key}")
                nc.scalar.activation(sio[:], gp[:, 0:3 * B], AF.Sigmoid)
                tg = sbt.tile([128, B], f32, tag=f"tg{key}", name=f"tg{key}")
                nc.scalar.activation(tg[:], gp[:, 3 * B:4 * B], AF.Tanh)
                fc = sbt.tile([128, B], f32, tag=f"fc{key}", name=f"fc{key}")
                nc.vector.tensor_mul(out=fc[:], in0=sio[:, B:2 * B], in1=c[:])
                ig = sbt.tile([128, B], f32, tag=f"ig{key}", name=f"ig{key}")
                nc.vector.tensor_mul(out=ig[:], in0=sio[:, 0:B], in1=tg[:])
                nc.vector.tensor_add(out=c[:], in0=fc[:], in1=ig[:])
                tc_ = sbt.tile([128, B], f32, tag=f"tc{key}", name=f"tc{key}")
                nc.scalar.activation(tc_[:], c[:], AF.Tanh)
                nc.vector.tensor_mul(out=sb[:, off:off + B],
                                     in0=sio[:, 2 * B:3 * B], in1=tc_[:])

            def rec_mms(gp, cons, src_off, wname, first, last):
                """8 k-tiles x 4 types of [128,128]x[128,B] accumulation."""
                for kk in range(8):
                    for gi in range(4):
                        nc.tensor.matmul(
                            gp[:, gi * B:(gi + 1) * B],
                            W[wname][:, kk, gi], cons[:, kk, src_off:src_off + B],
                            start=False, stop=(last and kk == 7),
                            skip_group_check=True)

            cur_grp = [None]

            for rep_s in range(reps * NSLOT):
                s = rep_s % NSLOT
                rep = rep_s // NSLOT
                if s == 0:
                    for ct in cts:
                        nc.vector.memset(ct[:], 0.0)
                ex_base = rep * NEX
                cons = Hgp[(ex_base + s - 1) % NB]  # exchange issued at slot s-1
                l1_active = s < t_steps
                t2, t3 = s - 1, s - 2
                l2_active = 0 <= t2 < t_steps
                l3_active = 0 <= t3 < t_steps
                do_ex = s <= t_steps + 1
                if do_ex:
                    sb = stg[(ex_base + s) % 2]

                # ---------- L2 matmuls (consume freshest exchange) ----------
                if l2_active:
                    g2 = gps.tile([128, 4 * B], f32, tag="g", name="g2")
                    nc.tensor.matmul(g2[:], b2f[:], oh4[:], start=True,
                                     stop=False, skip_group_check=True)
                    rec_mms(g2, cons, 0, "Wih2", False, t2 == 0)
                    if t2 > 0:
                        rec_mms(g2, cons, B, "Whh2", False, True)

                # ---------- L3 matmuls ----------
                if l3_active:
                    g3 = gps.tile([128, 4 * B], f32, tag="g", name="g3")
                    nc.tensor.matmul(g3[:], b3f[:], oh4[:], start=True,
                                     stop=False, skip_group_check=True)
                    rec_mms(g3, cons, B, "Wih3", False, t3 == 0)
                    if t3 > 0:
                        rec_mms(g3, cons, 2 * B, "Whh3", False, True)

                if l2_active:
                    lstm_ew("2", g2, cts[1], sb, B)
                if l3_active:
                    lstm_ew("3", g3, cts[2], sb, 2 * B)

                # ---------- L1 (x-side independent; rec consumes h1) -------
                if l1_active:
                    xs = xq.tile([128, B], bf16)
                    nc.sync.dma_start(xs[:], p_Xt[s])
                    g1 = gps.tile([128, 4 * B], f32, tag="g", name="g1")
                    for gi in range(4):
                        nc.tensor.matmul(g1[:, gi * B:(gi + 1) * B],
                                         W1xf[:, gi], xs[:], start=True,
                                         stop=(s == 0), skip_group_check=True)
                    if s > 0:
                        rec_mms(g1, cons, 0, "Whh1", False, True)
                    lstm_ew("1", g1, cts[0], sb, 0)

                # ---------- exchange ----------
                if do_ex:
                    agin = dms.tile([128, W23], bf16, tag="agin", name="agin")
                    nc.sync.dma_start(agin[:], sb[:])
                    agout = dms.tile([R, 128, W23], bf16, tag="agout", name="agout")
                    nc.gpsimd.collective_compute(
                        "AllGather", mybir.AluOpType.bypass,
                        replica_groups=rg, ins=[agin[:].opt()], outs=[agout[:].opt()],
                    )
                    nxt = Hgp[(ex_base + s) % NB]
                    # h1 part lands first (it heads the next slot's recurrence);
                    # h2/h3 parts follow on a different queue
                    nc.gpsimd.dma_start(
                        nxt[:, :, 0:B],
                        agout[:, :, 0:B].rearrange("r p w -> p r w"))
                    nc.scalar.dma_start(
                        nxt[:, :, B:3 * B],
                        agout[:, :, B:3 * B].rearrange("r p w -> p r w"))

                # ---------- h3 group copy (t = s-3) + projection ----------
                tg3 = s - 3
                if 0 <= tg3 < t_steps:
                    gidx, j = tg3 // YG, tg3 % YG
                    if j == 0:
                        cur_grp[0] = h3g.tile([128, 8, YG, B], bf16,
                                              tag="h3grp", name="h3grp")
                    grp = cur_grp[0]
                    nc.sync.dma_start(grp[:, :, j, :], cons[:, :, 2 * B:3 * B])
                    if j == YG - 1:
                        yp = yps.tile([F, YG * B], f32)
                        for k in range(8):
                            nc.tensor.matmul(yp[:], Wout[:, k], grp[:, k],
                                             start=(k == 0), stop=(k == 7))
                        ysb = sbt.tile([F, YG * B], f32, tag="ysb")
                        nc.scalar.activation(ysb[:], yp[:], AF.Tanh, bias=bout[:])
                        y0 = 0 if y_small else gidx * YG * B
                        nc.sync.dma_start(p_Y[:, y0:y0 + YG * B], ysb[:])

    nc.compile()
    return nc


_CACHED = {}


def _get_nc(t_steps=T):
    if t_steps not in _CACHED:
        _CACHED[t_steps] = build_nc(t_steps)
    return _CACHED[t_steps]


def make_in_maps(X, weights):
    return [_prep_core_inputs(r, X, weights) for r in range(R)]


def _weights_tuple(kw):
    return tuple(
        np.asarray(kw[k], np.float32)
        for k in ("w_ih1", "w_hh1", "b_ih1", "b_hh1", "w_ih2", "w_hh2", "b_ih2",
                  "b_hh2", "w_ih3", "w_hh3", "b_ih3", "b_hh3", "w_out", "b_out")
    )


def assemble_output(Y, t_steps=T):
    """[F, t*B] -> [B, t, F]"""
    return np.ascontiguousarray(Y.reshape(F, t_steps, B).transpose(2, 1, 0))


def kernel(X, **kw):
    from concourse.bass_utils import run_bass_kernel_spmd

    nc = _get_nc(T)
    in_maps = make_in_maps(np.asarray(X, np.float32), _weights_tuple(kw))
    res = run_bass_kernel_spmd(nc, in_maps, core_ids=list(range(R)))
    return assemble_output(res.results[0]["Y"])
